# revision 1
# baseline (speedup 1.0000x reference)
"""Distributed 2-layer GCN (PyG GCNConv semantics) on 8 TRN2 NeuronCores.

Strategy: nodes are permuted into degree-balanced tiles of 128 and sharded
across 8 cores (49 tiles / 6272 nodes per core).  Self-loops are appended as
ordinary edges.  Each conv layer builds a bf16 node-feature table in DRAM
(replicated computation for layer 1, AllGather for layer 2), gathers edge
messages with dma_gather (int16 indices, lo/hi table halves), and scatter-adds
them with one-hot selection-matrix matmuls accumulating in PSUM.  Degrees are
computed on-device with a ones-vector matmul over the same selection matrices.
"""

import math

import numpy as np
import ml_dtypes

import concourse.bacc as bacc
import concourse.mybir as mybir
import concourse.tile as tile


class _Runner:
    """Cached PJRT executable for a compiled Bass module (axon path).

    Mirrors concourse.bass2jax.run_bass_via_pjrt but keeps the jitted
    callable and device-resident inputs so repeated calls skip retracing
    and host->device transfers (useful for timing).
    """

    def __init__(self, nc, n_cores):
        import jax
        from jax.experimental.shard_map import shard_map
        from jax.sharding import Mesh, PartitionSpec
        from concourse import bass2jax

        bass2jax.install_neuronx_cc_hook()
        self.nc = nc
        self.n_cores = n_cores
        self.jax = jax
        partition_name = (nc.partition_id_tensor.name
                          if nc.partition_id_tensor else None)
        in_names, out_names, out_avals, zero_outs = [], [], [], []
        for alloc in nc.m.functions[0].allocations:
            if not isinstance(alloc, mybir.MemoryLocationSet):
                continue
            name = alloc.memorylocations[0].name
            if alloc.kind == "ExternalInput":
                if name != partition_name:
                    in_names.append(name)
            elif alloc.kind == "ExternalOutput":
                shape = tuple(alloc.tensor_shape)
                dtype = mybir.dt.np(alloc.dtype)
                out_names.append(name)
                out_avals.append(jax.core.ShapedArray(shape, dtype))
                zero_outs.append(np.zeros(shape, dtype))
        self.in_names = list(in_names)
        self.out_names = out_names
        self.zero_outs = zero_outs
        n_params = len(in_names)
        n_outs = len(out_avals)
        all_names = in_names + out_names
        if partition_name is not None:
            all_names.append(partition_name)
        donate = tuple(range(n_params, n_params + n_outs))

        def _body(*args):
            operands = list(args)
            if partition_name is not None:
                operands.append(bass2jax.partition_id_tensor())
            outs = bass2jax._bass_exec_p.bind(
                *operands,
                out_avals=tuple(out_avals),
                in_names=tuple(all_names),
                out_names=tuple(out_names),
                lowering_input_output_aliases=(),
                sim_require_finite=True,
                sim_require_nnan=True,
                nc=nc,
            )
            return tuple(outs)

        devices = jax.devices()[:n_cores]
        self.mesh = Mesh(np.asarray(devices), ("core",))
        self.spec = PartitionSpec("core")
        in_specs = (self.spec,) * (n_params + n_outs)
        out_specs = (self.spec,) * n_outs
        self.fn = jax.jit(
            shard_map(_body, mesh=self.mesh, in_specs=in_specs,
                      out_specs=out_specs, check_rep=False),
            donate_argnums=donate, keep_unused=True,
        )
        self.dev_inputs = None

    def put_inputs(self, in_maps):
        import jax
        from jax.sharding import NamedSharding
        sh = NamedSharding(self.mesh, self.spec)
        self.dev_inputs = [
            jax.device_put(
                np.concatenate([np.asarray(in_maps[c][n])
                                for c in range(self.n_cores)], axis=0), sh)
            for n in self.in_names
        ]

    def run(self):
        outs = self.fn(*self.dev_inputs,
                       *[np.concatenate([z] * self.n_cores, axis=0)
                         for z in self.zero_outs])
        return [
            {n: np.asarray(outs[i]).reshape(self.n_cores, *self.zero_outs[i].shape)[c]
             for i, n in enumerate(self.out_names)}
            for c in range(self.n_cores)
        ]

    def time_rounds(self, rounds):
        """Issue `rounds` back-to-back executions; return per-round seconds
        from the async-pipelined tail (excludes dispatch latency and
        host->device transfer of the donated output buffers)."""
        import time
        import jax
        from jax.sharding import NamedSharding
        sh = NamedSharding(self.mesh, self.spec)
        zo = [[jax.device_put(np.concatenate([z] * self.n_cores, axis=0), sh)
               for z in self.zero_outs] for _ in range(rounds + 1)]
        jax.block_until_ready(zo)
        r = self.fn(*self.dev_inputs, *zo[0])
        self.jax.block_until_ready(r)
        t0 = time.perf_counter()
        last = None
        for i in range(rounds):
            last = self.fn(*self.dev_inputs, *zo[i + 1])
        self.jax.block_until_ready(last)
        t1 = time.perf_counter()
        return (t1 - t0) / rounds

# problem shape (hardcoded per spec)
N = 50000
D = 128
K = 100
N_CORES = 8

T_PER_CORE = 49
T = T_PER_CORE * N_CORES          # 392 tiles
NP = T * 128                      # 50176 padded nodes
NLOC = T_PER_CORE * 128           # 6272 nodes per core
HALF = NP // 2                    # 25088: lo/hi split for int16 indices
GROUP = 7                         # tiles per dma_gather call
NGROUPS = T_PER_CORE // GROUP     # 7 groups per core

F32 = mybir.dt.float32
BF16 = mybir.dt.bfloat16
I16 = mybir.dt.int16
AF = mybir.ActivationFunctionType
ALU = mybir.AluOpType

_CACHE = {}
_LAST_RUNNER = None
LEVEL = 99  # debug bisect: 1=deg, 2=+g1, 3=+conv1, 4=+interlude, 5=full


# ----------------------------------------------------------------------------
# host-side preprocessing: permutation, edge bucketing, index streams
# ----------------------------------------------------------------------------

def _pack_tiles(src0, dst0):
    """Two-phase permutation: (1) randomly split nodes into lo/hi regions
    (fixing every edge's src half, hence each node's (lo,hi) in-degree);
    (2) per region, 2-D greedy bin-packing of nodes into 128-slot tiles
    minimizing max(lo_edges, hi_edges) per tile -- this bounds the chunk
    count NCH.  Returns orig_of_new[NP]."""
    rng = np.random.default_rng(12345)
    region = np.zeros(NP, np.int8)
    region[rng.permutation(NP)[HALF:]] = 1
    lo_in = np.bincount(dst0[region[src0] == 0], minlength=NP).astype(np.int64)
    hi_in = np.bincount(dst0[region[src0] == 1], minlength=NP).astype(np.int64)
    lo_in += region == 0  # self loop
    hi_in += region == 1

    orig_of_new = np.empty(NP, np.int64)
    half_tiles = T // 2
    for r in (0, 1):
        nodes = np.where(region == r)[0]
        order = np.argsort(-(lo_in[nodes] + hi_in[nodes]), kind="stable")
        nodes = nodes[order]
        lo_t = np.zeros(half_tiles, np.float64)
        hi_t = np.zeros(half_tiles, np.float64)
        cnt = np.zeros(half_tiles, np.int64)
        big = 1e18
        for n in nodes:
            score = np.maximum(lo_t + lo_in[n], hi_t + hi_in[n])
            score[cnt >= 128] = big
            t = int(np.argmin(score))
            gt = r * half_tiles + t
            orig_of_new[gt * 128 + cnt[t]] = n
            lo_t[t] += lo_in[n]
            hi_t[t] += hi_in[n]
            cnt[t] += 1
    return orig_of_new


def _preprocess(edge_index):
    src0 = np.asarray(edge_index[0], np.int64)
    dst0 = np.asarray(edge_index[1], np.int64)

    orig_of_new = _pack_tiles(src0, dst0)
    new_of_orig = np.empty(NP, np.int64)
    new_of_orig[orig_of_new] = np.arange(NP)

    # edges in new ids, plus self-loops for every padded node
    src = np.concatenate([new_of_orig[src0], np.arange(NP)])
    dst = np.concatenate([new_of_orig[dst0], np.arange(NP)])

    tile_id = dst >> 7
    dstloc = (dst & 127).astype(np.uint8)
    half = (src >= HALF).astype(np.int64)
    idx16 = (src - half * HALF).astype(np.int16)

    seg = tile_id * 2 + half
    seg_counts = np.bincount(seg, minlength=2 * T)
    nch = math.ceil(seg_counts.max() / 128)
    slots = nch * 128

    order = np.argsort(seg, kind="stable")
    seg_sorted = seg[order]
    seg_start = np.zeros(2 * T + 1, np.int64)
    np.cumsum(seg_counts, out=seg_start[1:])
    pos = np.arange(len(order)) - seg_start[seg_sorted]

    idx_stream = np.zeros((T, 2, slots), np.int16)
    dst_stream = np.full((T, 2, slots), 255, np.uint8)
    e_tile = tile_id[order]
    e_half = half[order]
    idx_stream[e_tile, e_half, pos] = idx16[order]
    dst_stream[e_tile, e_half, pos] = dstloc[order]

    # per-core tensors
    per_core = []
    for m in range(N_CORES):
        ts = slice(m * T_PER_CORE, (m + 1) * T_PER_CORE)
        # dstloc: [128, T_PER_CORE * 2*nch] bf16, chunk-major per tile
        dsl = dst_stream[ts].reshape(T_PER_CORE, 2 * nch, 128)
        dsl = np.ascontiguousarray(dsl.transpose(2, 0, 1)).reshape(128, -1)
        dsl = dsl.astype(np.float32).astype(ml_dtypes.bfloat16)
        # gather index streams: per half, groups of GROUP tiles
        idxs = []
        for h in range(2):
            v = idx_stream[ts, h].reshape(NGROUPS, GROUP * slots)
            cols = []
            for g in range(NGROUPS):
                a = v[g].reshape(-1, 16).T  # [16, L/16]
                cols.append(np.tile(a, (8, 1)))
            idxs.append(np.ascontiguousarray(np.concatenate(cols, axis=1)))
        per_core.append((dsl, idxs[0], idxs[1]))
    return orig_of_new, nch, per_core


# ----------------------------------------------------------------------------
# device program
# ----------------------------------------------------------------------------

def _build(nch):
    nc = bacc.Bacc("TRN2", target_bir_lowering=False, debug=False,
                   num_devices=N_CORES)
    nch2 = 2 * nch
    slots = nch * 128
    gidx_cols = GROUP * slots // 16

    xT = nc.dram_tensor("xT", [D, NLOC], F32, kind="ExternalInput")
    w1 = nc.dram_tensor("w1", [D, D], F32, kind="ExternalInput")
    w2 = nc.dram_tensor("w2", [D, D], F32, kind="ExternalInput")
    wa = nc.dram_tensor("wa", [D, K], F32, kind="ExternalInput")
    b1m = nc.dram_tensor("b1m", [128, D], F32, kind="ExternalInput")
    b2m = nc.dram_tensor("b2m", [128, D], F32, kind="ExternalInput")
    bam = nc.dram_tensor("bam", [128, K], F32, kind="ExternalInput")
    identm = nc.dram_tensor("identm", [128, 128], F32, kind="ExternalInput")
    iotam = nc.dram_tensor("iotam", [128, nch2 * 128], F32, kind="ExternalInput")
    dslm = nc.dram_tensor("dslm", [128, T_PER_CORE * nch2], BF16, kind="ExternalInput")
    idxlo = nc.dram_tensor("idxlo", [128, NGROUPS * gidx_cols], I16, kind="ExternalInput")
    idxhi = nc.dram_tensor("idxhi", [128, NGROUPS * gidx_cols], I16, kind="ExternalInput")
    out = nc.dram_tensor("out", [NLOC, K], F32, kind="ExternalOutput")

    g1_tab = nc.dram_tensor("g1_tab", [NP, D], BF16, addr_space="Shared")
    g2_tab = nc.dram_tensor("g2_tab", [NP, D], BF16, addr_space="Shared")
    g1_in = nc.dram_tensor("g1_in", [NLOC, D], BF16)
    deg_dram = nc.dram_tensor("deg_dram", [NLOC], F32)
    g2_in = nc.dram_tensor("g2_in", [NLOC, D], BF16)

    groups = [list(range(N_CORES))]

    with tile.TileContext(nc) as tc:
        with tc.tile_pool(name="res", bufs=1) as res:
            def body():
                # ---- resident constants / streams ----
                dsl_sb = res.tile([128, T_PER_CORE * nch2], BF16, tag="dsl")
                nc.sync.dma_start(out=dsl_sb[:], in_=dslm[:])
                ixlo_sb = res.tile([128, NGROUPS * gidx_cols], I16, tag="ixlo")
                nc.sync.dma_start(out=ixlo_sb[:], in_=idxlo[:])
                ixhi_sb = res.tile([128, NGROUPS * gidx_cols], I16, tag="ixhi")
                nc.sync.dma_start(out=ixhi_sb[:], in_=idxhi[:])
                iota_sb = res.tile([128, nch2 * 128], BF16, tag="iota")
                nc.gpsimd.dma_start(out=iota_sb[:], in_=iotam[:])  # f32 -> bf16
                ident_sb = res.tile([128, 128], F32, tag="ident")
                nc.sync.dma_start(out=ident_sb[:], in_=identm[:])
                identb_sb = res.tile([128, 128], BF16, tag="identb")
                nc.gpsimd.dma_start(out=identb_sb[:], in_=identm[:])
                w1_sb = res.tile([128, D], BF16, tag="w1")
                nc.gpsimd.dma_start(out=w1_sb[:], in_=w1[:])
                w2_sb = res.tile([128, D], BF16, tag="w2")
                nc.gpsimd.dma_start(out=w2_sb[:], in_=w2[:])
                wa_sb = res.tile([128, K], F32, tag="wa")
                nc.sync.dma_start(out=wa_sb[:], in_=wa[:])
                b1_sb = res.tile([128, D], F32, tag="b1")
                nc.sync.dma_start(out=b1_sb[:], in_=b1m[:])
                b2_sb = res.tile([128, D], F32, tag="b2")
                nc.sync.dma_start(out=b2_sb[:], in_=b2m[:])
                ba_sb = res.tile([128, K], F32, tag="ba")
                nc.sync.dma_start(out=ba_sb[:], in_=bam[:])
                ones_sb = res.tile([128, 1], BF16, tag="ones")
                nc.vector.memset(ones_sb[:], 1.0)
                dinv_own = res.tile([128, T_PER_CORE], F32, tag="dinv_own")
                r1_sb = res.tile([128, NLOC], BF16, tag="r1")

                def build_s(spool, t):
                    s = spool.tile([128, nch2, 128], BF16, tag="S")
                    nc.vector.tensor_tensor(
                        out=s[:],
                        in0=dsl_sb[:, t * nch2:(t + 1) * nch2].unsqueeze(2)
                            .to_broadcast([128, nch2, 128]),
                        in1=iota_sb[:].rearrange("p (c j) -> p c j", j=128),
                        op=ALU.is_equal,
                    )
                    return s

                # ---- phase A: degree pass -> dinv (own tiles only) ----
                with (
                    tc.tile_pool(name="dega", bufs=3) as dega,
                    tc.tile_pool(name="degp", bufs=4, space="PSUM") as degp,
                ):
                    wide = res.tile([1, NLOC], F32, tag="degwide")
                    for t in range(T_PER_CORE):
                        s = build_s(dega, t)
                        pd = degp.tile([1, 128], F32, tag="pd")
                        for c in range(nch2):
                            nc.tensor.matmul(
                                out=pd[:], lhsT=ones_sb[:], rhs=s[:, c, :],
                                start=(c == 0), stop=(c == nch2 - 1),
                            )
                        nc.vector.tensor_copy(
                            out=wide[0:1, t * 128:(t + 1) * 128], in_=pd[:])
                    degsq = dega.tile([128, 128], F32, tag="degsq")
                    nc.vector.memset(degsq[:], 1.0)
                    nc.sync.dma_start(out=deg_dram[None, :], in_=wide[0:1, :])
                    nc.sync.dma_start(
                        out=degsq[0:T_PER_CORE, :],
                        in_=deg_dram[:].rearrange("(t j) -> t j", j=128),
                    )
                    ptr = degp.tile([128, 128], F32, tag="ptr")
                    nc.tensor.transpose(out=ptr[:], in_=degsq[:],
                                        identity=ident_sb[:])
                    sq = dega.tile([128, T_PER_CORE], F32, tag="sq")
                    nc.scalar.activation(sq[:], ptr[:, :T_PER_CORE], AF.Sqrt)
                    nc.vector.reciprocal(out=dinv_own[:], in_=sq[:])

                if LEVEL <= 1:
                    dmp = res.tile([128, K], F32, tag="dmp")
                    nc.vector.memset(dmp[:], 0.0)
                    nc.vector.tensor_copy(out=dmp[:, :T_PER_CORE], in_=dinv_own[:])
                    nc.sync.dma_start(out=out[0:128, :], in_=dmp[:])
                    return

                # ---- phase B: g1_own = dinv * (x @ W1); AllGather table ----
                XB = 7 if T_PER_CORE % 7 == 0 else 1  # tiles per staged chunk
                with (
                    tc.tile_pool(name="xb", bufs=3) as xb,
                    tc.tile_pool(name="g1c", bufs=3) as g1c,
                    tc.tile_pool(name="hp", bufs=6, space="PSUM") as hpp,
                ):
                    for ch in range(T_PER_CORE // XB):
                        xc = xb.tile([128, XB * 128], BF16, tag="xc")
                        nc.gpsimd.dma_start(
                            out=xc[:], in_=xT[:, ch * XB * 128:(ch + 1) * XB * 128]
                        )
                        gc = g1c.tile([128, XB, 128], BF16, tag="gc")
                        for j in range(XB):
                            t = ch * XB + j
                            hp = hpp.tile([128, 128], F32, tag="hp")
                            nc.tensor.matmul(
                                out=hp[:], lhsT=xc[:, j * 128:(j + 1) * 128],
                                rhs=w1_sb[:], start=True, stop=True,
                            )
                            nc.scalar.activation(
                                gc[:, j, :], hp[:], AF.Copy,
                                scale=dinv_own[:, t:t + 1],
                            )
                        nc.sync.dma_start(
                            out=g1_in[ch * XB * 128:(ch + 1) * XB * 128, :]
                                .rearrange("(c p) f -> p c f", p=128),
                            in_=gc[:],
                        )
                    nc.gpsimd.collective_compute(
                        "AllGather", ALU.bypass, replica_groups=groups,
                        ins=[g1_in[:]], outs=[g1_tab[:]],
                    )

                if LEVEL <= 2:
                    dmp = res.tile([128, K], F32, tag="dmp")
                    nc.gpsimd.dma_start(out=dmp[:], in_=g1_tab[0:128, 0:K])
                    nc.sync.dma_start(out=out[0:128, :], in_=dmp[:])
                    return

                # ---- conv pass (shared for layer 1 and 2) ----
                def conv(tab, evict):
                    with (
                        tc.tile_pool(name="mbuf", bufs=1) as mpool,
                        tc.tile_pool(name="sbuf", bufs=3) as spool,
                        tc.tile_pool(name="ebuf", bufs=3) as epool,
                        tc.tile_pool(name="accp", bufs=4, space="PSUM") as accp,
                    ):
                        for g in range(NGROUPS):
                            mlo = mpool.tile([128, GROUP * nch, 128], BF16, tag="mlo")
                            mhi = mpool.tile([128, GROUP * nch, 128], BF16, tag="mhi")
                            nc.gpsimd.dma_gather(
                                out_ap=mlo[:], in_ap=tab[0:HALF, :],
                                idxs_ap=ixlo_sb[:, g * gidx_cols:(g + 1) * gidx_cols],
                                num_idxs=GROUP * slots, num_idxs_reg=GROUP * slots,
                                elem_size=D, single_packet=False,
                            )
                            nc.gpsimd.dma_gather(
                                out_ap=mhi[:], in_ap=tab[HALF:NP, :],
                                idxs_ap=ixhi_sb[:, g * gidx_cols:(g + 1) * gidx_cols],
                                num_idxs=GROUP * slots, num_idxs_reg=GROUP * slots,
                                elem_size=D, single_packet=False,
                            )
                            for tw in range(GROUP):
                                t = g * GROUP + tw
                                s = build_s(spool, t)
                                acc = accp.tile([128, 128], F32, tag="acc")
                                for c in range(nch2):
                                    m = mlo if c < nch else mhi
                                    cc = c if c < nch else c - nch
                                    nc.tensor.matmul(
                                        out=acc[:], lhsT=s[:, c, :],
                                        rhs=m[:, tw * nch + cc, :],
                                        start=(c == 0), stop=(c == nch2 - 1),
                                    )
                                evict(epool, t, acc)

                # conv1 evict: r1 = relu(dinv*acc + b1) -> r1_sb bf16
                def evict1(epool, t, acc):
                    tmp = epool.tile([128, 128], F32, tag="tmp")
                    nc.vector.scalar_tensor_tensor(
                        out=tmp[:], in0=acc[:], scalar=dinv_own[:, t:t + 1],
                        in1=b1_sb[:], op0=ALU.mult, op1=ALU.add,
                    )
                    nc.scalar.activation(r1_sb[:, t * 128:(t + 1) * 128], tmp[:],
                                         AF.Relu)

                conv(g1_tab, evict1)

                if LEVEL <= 3:
                    dmp = res.tile([128, K], F32, tag="dmp")
                    nc.vector.tensor_copy(out=dmp[:], in_=r1_sb[:, 0:K])
                    nc.sync.dma_start(out=out[0:128, :], in_=dmp[:])
                    return

                # ---- interlude: g2_own = dinv * (r1 @ W2); AllGather ----
                with (
                    tc.tile_pool(name="ibuf", bufs=3) as ibuf,
                    tc.tile_pool(name="ip", bufs=4, space="PSUM") as ipp,
                ):
                    for t in range(T_PER_CORE):
                        trp = ipp.tile([128, 128], BF16, tag="trp")
                        nc.tensor.transpose(
                            out=trp[:], in_=r1_sb[:, t * 128:(t + 1) * 128],
                            identity=identb_sb[:],
                        )
                        r1t = ibuf.tile([128, 128], BF16, tag="r1t")
                        nc.vector.tensor_copy(out=r1t[:], in_=trp[:])
                        t2p = ipp.tile([128, 128], F32, tag="t2p")
                        nc.tensor.matmul(out=t2p[:], lhsT=r1t[:], rhs=w2_sb[:],
                                         start=True, stop=True)
                        g2c = ibuf.tile([128, 128], BF16, tag="g2c")
                        nc.scalar.activation(g2c[:], t2p[:], AF.Copy,
                                             scale=dinv_own[:, t:t + 1])
                        nc.sync.dma_start(
                            out=g2_in[t * 128:(t + 1) * 128, :], in_=g2c[:]
                        )
                    nc.gpsimd.collective_compute(
                        "AllGather", ALU.bypass, replica_groups=groups,
                        ins=[g2_in[:]], outs=[g2_tab[:]],
                    )

                if LEVEL <= 4:
                    dmp = res.tile([128, K], F32, tag="dmp")
                    nc.gpsimd.dma_start(out=dmp[:], in_=g2_tab[0:128, 0:K])
                    nc.sync.dma_start(out=out[0:128, :], in_=dmp[:])
                    return

                # conv2 evict: a2 = dinv*acc + b2; logits; softmax; store
                with (
                    tc.tile_pool(name="fbuf", bufs=3) as fbuf,
                    tc.tile_pool(name="fp", bufs=2, space="PSUM") as fpp,
                ):
                    def evict2(epool, t, acc):
                        a2 = epool.tile([128, 128], F32, tag="a2")
                        nc.vector.scalar_tensor_tensor(
                            out=a2[:], in0=acc[:], scalar=dinv_own[:, t:t + 1],
                            in1=b2_sb[:], op0=ALU.mult, op1=ALU.add,
                        )
                        trp = fpp.tile([128, 128], F32, tag="trp2")
                        nc.tensor.transpose(out=trp[:], in_=a2[:],
                                            identity=ident_sb[:])
                        a2t = fbuf.tile([128, 128], F32, tag="a2t")
                        nc.vector.tensor_copy(out=a2t[:], in_=trp[:])
                        lgp = fpp.tile([128, K], F32, tag="lgp")
                        nc.tensor.matmul(out=lgp[:], lhsT=a2t[:], rhs=wa_sb[:],
                                         start=True, stop=True)
                        lg = fbuf.tile([128, K], F32, tag="lg")
                        nc.vector.tensor_tensor(out=lg[:], in0=lgp[:], in1=ba_sb[:],
                                                op=ALU.add)
                        nmx = fbuf.tile([128, 1], F32, tag="nmx")
                        nc.vector.reduce_max(out=nmx[:], in_=lg[:],
                                             axis=mybir.AxisListType.X, negate=True)
                        ex = fbuf.tile([128, K], F32, tag="ex")
                        nc.scalar.activation(ex[:], lg[:], AF.Exp, bias=nmx[:])
                        sm = fbuf.tile([128, 1], F32, tag="sm")
                        nc.vector.reduce_sum(out=sm[:], in_=ex[:],
                                             axis=mybir.AxisListType.X)
                        rc = fbuf.tile([128, 1], F32, tag="rc")
                        nc.vector.reciprocal(out=rc[:], in_=sm[:])
                        ot = fbuf.tile([128, K], F32, tag="ot")
                        nc.vector.tensor_scalar(
                            out=ot[:], in0=ex[:], scalar1=rc[:], scalar2=None,
                            op0=ALU.mult,
                        )
                        nc.sync.dma_start(out=out[t * 128:(t + 1) * 128, :],
                                          in_=ot[:])

                    conv(g2_tab, evict2)

            body()

    nc.compile()
    return nc


# ----------------------------------------------------------------------------
# entry point
# ----------------------------------------------------------------------------

def kernel(x, edge_index, W1, b1, W2, b2, Wa, ba):
    x = np.asarray(x, np.float32)
    W1 = np.asarray(W1, np.float32)
    W2 = np.asarray(W2, np.float32)
    Wa = np.asarray(Wa, np.float32)
    b1 = np.asarray(b1, np.float32)
    b2 = np.asarray(b2, np.float32)
    ba = np.asarray(ba, np.float32)

    orig_of_new, nch, per_core = _preprocess(edge_index)

    if nch not in _CACHE:
        _CACHE[nch] = _Runner(_build(nch), N_CORES)
    runner = _CACHE[nch]

    # xp[new] = x[orig] for real nodes, zeros for padding
    xp = np.zeros((NP, D), np.float32)
    mask = orig_of_new < N
    xp[np.arange(NP)[mask]] = x[orig_of_new[mask]]
    xT = np.ascontiguousarray(xp.T)

    iota = np.broadcast_to(
        np.tile(np.arange(128, dtype=np.float32), 2 * nch)[None, :],
        (128, 2 * nch * 128),
    ).copy()
    ident = np.eye(128, dtype=np.float32)
    b1m = np.broadcast_to(b1[None, :], (128, D)).copy()
    b2m = np.broadcast_to(b2[None, :], (128, D)).copy()
    bam = np.broadcast_to(ba[None, :], (128, K)).copy()

    in_maps = []
    for m in range(N_CORES):
        dsl, ixlo, ixhi = per_core[m]
        in_maps.append({
            "xT": np.ascontiguousarray(xT[:, m * NLOC:(m + 1) * NLOC]),
            "w1": W1, "w2": W2, "wa": Wa,
            "b1m": b1m, "b2m": b2m, "bam": bam,
            "identm": ident, "iotam": iota,
            "dslm": dsl, "idxlo": ixlo, "idxhi": ixhi,
        })

    runner.put_inputs(in_maps)
    results = runner.run()
    global _LAST_RUNNER
    _LAST_RUNNER = runner

    full = np.concatenate([results[m]["out"] for m in range(N_CORES)], axis=0)
    final = np.empty((N, K), np.float32)
    final[orig_of_new[mask]] = full[mask]
    return final

